# revision 12
# baseline (speedup 1.0000x reference)
"""GCN encoder (2x GCNConv + ReLU + AdaptiveAvgPool) on 8 Trainium2 NeuronCores.

Math (matches reference):
    deg[i]  = #edges with dst==i (+1 self loop);  dinv = deg^-1/2
    h       = relu( A_norm @ (x @ W1) + b1 ),  A_norm = D^-1/2 (A+I) D^-1/2
    out2    = A_norm @ (h @ W2) + b2
    pooled[g] = mean over nodes n in group g (1600 nodes) of out2[n]

Key algebraic restructurings (exact, fp-reassociation only):
  * W1 commutes with aggregation: A_norm @ (x@W1) = (A_norm @ x) @ W1,
    so the per-edge payload is one 16-float x row, not 64.
  * A_norm factorizes: agg[d] = dinv[d] * sum_{e->d} xd[src_e],
    xd = dinv[:,None]*x.  No per-edge weights on device.
  * The pooled output only needs z[g] = sum_n C[n,g] * (dinv[n] h[n])
    with C[n,g] = sum_{e: src=n, dst in g} dinv[dst_e] (host-built
    graph metadata).  pooled = (z @ W2)/1600 + b2.

Gather strategy: dma_gather is limited to int16 indices and 256B-multiple
elements on 4 SWDGE queues (~22.5 GB/s each), so sources are quad-packed
(4 rows = 256B, idx = src>>2) and a host-baked per-slot mask
(dinv[dst] x onehot(src&3)) selects the right row during a DVE multiply +
free-axis segment reduce.  At ~817 slot columns the gather is at the
4-queue byte floor (~27 MB -> ~300 us); descriptors are the binding
resource, so:

  * Self-loops are removed from the gather stream: the host ships a
    per-core block-grid xdd[128, NBLK*F] = dinv[d]^2 * x[d] (sequential
    DMA) added to the per-block segment-reduce result on DVE
    (~5.6% fewer gather descriptors).
  * All gather index streams and slot masks are preloaded into SBUF in
    per-chunk slices ordered by first use (a monolithic preload DMA would
    stall the first gathers ~90us), so chunk gathers depend only on gbuf
    rotation (4 buffers, 16 gathers in flight across the 4 queues).
  * 28 gathers of 4096 descriptors (GCOLS=32) queue-balanced by LPT +
    local search to within ~2% of ideal (trailing -1 indices are skipped
    by the firmware via num_idxs_reg).
  * Chunks are processed largest-caps-first so the trailing DVE work
    lands on the smallest chunk.

Device work per core: ~26.8 MB of random 256B reads, mask-mult + segment
reduce on DVE, transpose, @W1+b1, relu*dinv, z psum-accum,
(z@W2+200*b2)/1600.  Host combines: output = sum of 8 partials.
"""

import numpy as np

N = 51200
E = 819200
F = 16
H1 = 64
H2 = 128
G = 32
GS = N // G
NCORES = 8
NPC = N // NCORES
NBLK = NPC // 128
PAD_IDX = 10_000_000
GCOLS = 32
CHUNK_G = 4
CHUNK_COLS = GCOLS * CHUNK_G


def _prep(x, edge_index, W1, b1, W2, b2):
    src = edge_index[0].astype(np.int64)
    dst = edge_index[1].astype(np.int64)

    deg_e = np.bincount(dst, minlength=N)
    deg = deg_e + 1
    dinv = (1.0 / np.sqrt(deg.astype(np.float64))).astype(np.float32)

    xd = (x.astype(np.float32) * dinv[:, None]).astype(np.float32)
    xdq = np.zeros((N // 4 + 1, 4 * F), np.float32)
    xdq[:N // 4] = xd.reshape(N // 4, 4 * F)
    xdd = xd * dinv[:, None]          # dinv^2 * x  (self-loop term)

    g_e = dst // GS
    C = np.bincount(src * G + g_e, weights=dinv[dst].astype(np.float64),
                    minlength=N * G).astype(np.float32).reshape(N, G)
    C[np.arange(N), np.arange(N) // GS] += dinv

    # dst-sorted source table (edges only, no self slot)
    order_e = np.argsort(dst, kind="stable")
    srcs_sorted = src[order_e].astype(np.int32)
    maxdeg_e = int(deg_e.max())
    Tw = ((maxdeg_e + 3) // 4) * 4
    T = np.full((N, Tw), PAD_IDX, np.int32)
    mask = np.arange(Tw)[None, :] < deg_e[:, None]
    T[mask] = srcs_sorted
    # sort each row's sources ascending (pads sort last): column k then
    # holds the k-th order statistic, so a gather column's 128 descriptors
    # cluster in a narrow table window -> better HBM locality across the
    # 4 contending SWDGE queues
    T.sort(axis=1)

    order_n = np.argsort(deg, kind="stable")
    cores_nodes = [order_n[c::NCORES] for c in range(NCORES)]

    caps = []
    for B in range(NBLK):
        m = 1
        for c in range(NCORES):
            nodes = cores_nodes[c][B * 128:(B + 1) * 128]
            m = max(m, int(deg_e[nodes].max()))
        caps.append(m)

    chunks = []
    col0, b0, acc = 0, 0, 0
    for B in range(NBLK):
        if acc + caps[B] > CHUNK_COLS:
            chunks.append((b0, B, col0))
            col0 += CHUNK_COLS
            b0, acc = B, 0
        acc += caps[B]
    chunks.append((b0, NBLK, col0))
    SP = col0 + CHUNK_COLS
    NG = SP // GCOLS
    boff = {}
    for (bb0, bb1, c0) in chunks:
        c = c0
        for B in range(bb0, bb1):
            boff[B] = c
            c += caps[B]

    w1 = np.ascontiguousarray(W1.astype(np.float32))
    b1r = np.ascontiguousarray(b1.astype(np.float32).reshape(1, H1))
    w2 = np.ascontiguousarray(W2.astype(np.float32))
    b2r = np.ascontiguousarray(b2.astype(np.float32).reshape(1, H2))

    per_core = []
    for c in range(NCORES):
        qidx_cols = np.full((128, SP), N // 4, np.int32)
        mask_all = np.zeros((128, SP, 4), np.float32)
        dinv_pos = np.zeros((128, NBLK), np.float32)
        c_all = np.zeros((128, NBLK * G), np.float32)
        xdd_all = np.zeros((128, NBLK * F), np.float32)
        for B in range(NBLK):
            nodes = cores_nodes[c][B * 128:(B + 1) * 128]
            cap = caps[B]
            st = T[nodes, :cap]
            pad = st == PAD_IDX
            o = boff[B]
            qidx_cols[:, o:o + cap] = np.where(pad, N // 4, st >> 2)
            sub = np.where(pad, 0, st & 3)
            m = (np.arange(4)[None, None, :] == sub[:, :, None]).astype(
                np.float32)
            m[pad] = 0.0
            m *= dinv[nodes][:, None, None]
            mask_all[:, o:o + cap, :] = m
            dinv_pos[:, B] = dinv[nodes]
            c_all[:, B * G:(B + 1) * G] = C[nodes]
            xdd_all[:, B * F:(B + 1) * F] = xdd[nodes]
        gidx = np.zeros((128, NG * (GCOLS * 128 // 16)), np.int16)
        W16 = GCOLS * 128 // 16
        vend = {}
        for (bb0, bb1, cc0) in chunks:
            vend[cc0 // CHUNK_COLS] = boff[bb1 - 1] + caps[bb1 - 1]
        gvalid = []
        for g in range(NG):
            nv = min(GCOLS, max(0, vend[g * GCOLS // CHUNK_COLS] - g * GCOLS))
            gvalid.append(nv * 128)
            pos = qidx_cols[:, g * GCOLS:(g + 1) * GCOLS]
            pv = pos.T.ravel().copy()
            pv[nv * 128:] = -1
            arr = pv.reshape(W16, 16).T.astype(np.int16)
            gidx[:, g * W16:(g + 1) * W16] = np.tile(arr, (8, 1))
        per_core.append(dict(
            xdq=xdq, gidx=gidx,
            mask_all=mask_all.reshape(128, SP * 4),
            dinv_pos=dinv_pos, c_all=c_all, xdd_all=xdd_all,
            w1=w1, b1r=b1r, w2=w2, b2r=b2r,
        ))

    cfg = (tuple(caps), tuple(chunks), SP, tuple(gvalid))
    return cfg, per_core


def _build(cfg, nrep=1):
    import concourse.bass as bass
    import concourse.bacc as bacc
    import concourse.tile as tile
    from concourse import mybir
    from concourse.masks import make_identity

    caps, chunks, SP, gvalid = cfg
    NG = SP // GCOLS
    W16 = GCOLS * 128 // 16
    boff = {}
    for (bb0, bb1, c0) in chunks:
        c = c0
        for B in range(bb0, bb1):
            boff[B] = c
            c += caps[B]

    # Queue balancing by descriptor count: LPT then single-move/swap local
    # search (queue choice is free per gather; the critical queue's serial
    # time bounds the gather phase).
    qload = [0] * 4
    qassign = {}
    for g in sorted(range(NG), key=lambda g: -gvalid[g]):
        if gvalid[g] == 0:
            continue
        q = min(range(4), key=lambda i: qload[i])
        qassign[g] = q
        qload[q] += gvalid[g]
    improved = True
    while improved:
        improved = False
        for g, q in list(qassign.items()):
            for q2 in range(4):
                if max(qload) > max(qload[q] - gvalid[g],
                                    qload[q2] + gvalid[g],
                                    *(qload[i] for i in range(4)
                                      if i not in (q, q2))):
                    qload[q] -= gvalid[g]
                    qload[q2] += gvalid[g]
                    qassign[g] = q
                    qassign[g] = q2
                    q = q2
                    improved = True
        for g1, q1 in list(qassign.items()):
            for g2, q2 in list(qassign.items()):
                if q1 == q2 or g1 >= g2:
                    continue
                n1 = qload[q1] - gvalid[g1] + gvalid[g2]
                n2 = qload[q2] - gvalid[g2] + gvalid[g1]
                if max(qload) > max(n1, n2, *(qload[i] for i in range(4)
                                              if i not in (q1, q2))):
                    qload[q1], qload[q2] = n1, n2
                    qassign[g1], qassign[g2] = q2, q1
                    q1 = qassign[g1]
                    improved = True

    f32 = mybir.dt.float32
    i16 = mybir.dt.int16

    nc = bacc.Bacc("TRN2", target_bir_lowering=False, debug=False,
                   num_devices=NCORES, num_swdge_queues=4)

    xdq_t = nc.dram_tensor("xdq", [N // 4 + 1, 4 * F], f32,
                           kind="ExternalInput")
    gidx_t = nc.dram_tensor("gidx", [128, NG * W16], i16,
                            kind="ExternalInput")
    mask_t = nc.dram_tensor("mask_all", [128, SP * 4], f32,
                            kind="ExternalInput")
    dinv_t = nc.dram_tensor("dinv_pos", [128, NBLK], f32,
                            kind="ExternalInput")
    c_t = nc.dram_tensor("c_all", [128, NBLK * G], f32, kind="ExternalInput")
    xdd_t = nc.dram_tensor("xdd_all", [128, NBLK * F], f32,
                           kind="ExternalInput")
    w1_t = nc.dram_tensor("w1", [F, H1], f32, kind="ExternalInput")
    b1_t = nc.dram_tensor("b1r", [1, H1], f32, kind="ExternalInput")
    w2_t = nc.dram_tensor("w2", [H1, H2], f32, kind="ExternalInput")
    b2_t = nc.dram_tensor("b2r", [1, H2], f32, kind="ExternalInput")
    out_t = nc.dram_tensor("p_out", [G, H2], f32, kind="ExternalOutput")

    AF = mybir.ActivationFunctionType
    AX = mybir.AxisListType
    OP = mybir.AluOpType

    with tile.TileContext(nc) as tc:
        with tc.tile_pool(name="const", bufs=1) as constp, \
             tc.tile_pool(name="stream", bufs=4) as streamp, \
             tc.tile_pool(name="work", bufs=3) as workp, \
             tc.tile_pool(name="psum", bufs=2, space="PSUM") as psump, \
             tc.tile_pool(name="psumacc", bufs=1, space="PSUM") as psumaccp:

            ident = constp.tile([128, 128], f32)
            make_identity(nc, ident[:])
            ones_row = constp.tile([1, 128], f32)
            nc.vector.memset(ones_row[:], 1.0)
            ones_b2 = constp.tile([1, G], f32)
            nc.vector.memset(ones_b2[:], float(GS) / NCORES)

            w1 = constp.tile([F, H1], f32)
            nc.sync.dma_start(out=w1[:], in_=w1_t[:, :])
            b1s = constp.tile([1, H1], f32)
            nc.sync.dma_start(out=b1s[:], in_=b1_t[:, :])
            w2 = constp.tile([H1, H2], f32)
            nc.sync.dma_start(out=w2[:], in_=w2_t[:, :])
            b2s = constp.tile([1, H2], f32)
            nc.sync.dma_start(out=b2s[:], in_=b2_t[:, :])
            dinv = constp.tile([128, NBLK], f32)
            nc.sync.dma_start(out=dinv[:], in_=dinv_t[:, :])
            # Preloaded gather indices and slot masks, sliced per chunk and
            # ordered by first use (largest-cap chunks run first so the
            # trailing DVE work lands on the smallest chunk) to avoid a
            # serialized multi-MB startup DMA blocking the first gathers.
            idxall = constp.tile([128, NG * W16], i16)
            maskall = constp.tile([128, SP * 4], f32)
            order = list(range(len(chunks)))[::-1]
            for ci in order:
                _, _, c0 = chunks[ci]
                gci = c0 // GCOLS
                nc.sync.dma_start(
                    out=idxall[:, gci * W16:(gci + CHUNK_G) * W16],
                    in_=gidx_t[:, gci * W16:(gci + CHUNK_G) * W16])
                nc.sync.dma_start(
                    out=maskall[:, c0 * 4:(c0 + CHUNK_COLS) * 4],
                    in_=mask_t[:, c0 * 4:(c0 + CHUNK_COLS) * 4])
                if ci == order[0]:
                    xddt = constp.tile([128, NBLK * F], f32)
                    nc.sync.dma_start(out=xddt[:], in_=xdd_t[:, :])
                    call = constp.tile([128, NBLK * G], f32)
                    nc.sync.dma_start(out=call[:], in_=c_t[:, :])

            psum_zT = psumaccp.tile([H1, G], f32)

            rep_chunks = [chunks[ci] for _ in range(nrep) for ci in order]
            for ci, (b0, b1_, c0) in enumerate(rep_chunks):
                gci = (c0 // GCOLS) % NG
                gchunk = streamp.tile([128, CHUNK_COLS * 4 * F], f32,
                                      tag="gbuf")
                for gi in range(CHUNK_G):
                    if gvalid[gci + gi] == 0:
                        continue
                    nc.gpsimd.dma_gather(
                        out_ap=gchunk[:, gi * GCOLS * 4 * F:
                                      (gi + 1) * GCOLS * 4 * F].rearrange(
                            "p (s f) -> p s f", f=4 * F),
                        in_ap=xdq_t[:, :],
                        idxs_ap=idxall[:, (gci + gi) * W16:
                                       (gci + gi + 1) * W16],
                        num_idxs=GCOLS * 128,
                        num_idxs_reg=gvalid[gci + gi],
                        elem_size=4 * F, single_packet=False,
                        queue_num=qassign[gci + gi])
                for B in range(b0, b1_):
                    o = boff[B] - c0
                    oc = boff[B]
                    cap = caps[B]
                    gv = gchunk[:, o * 4 * F:(o + cap) * 4 * F].rearrange(
                        "p (c j f) -> p c j f", j=4, f=F)
                    mv = maskall[:, oc * 4:(oc + cap) * 4].rearrange(
                        "p (c j) -> p c j", j=4)
                    mvb = bass.AP(mv.tensor, mv.offset,
                                  list(mv.ap) + [[0, F]])
                    nc.vector.tensor_tensor(out=gv, in0=gv, in1=mvb,
                                            op=OP.mult)
                    agg = workp.tile([128, F], f32, tag="agg")
                    nc.vector.tensor_reduce(
                        agg[:],
                        gchunk[:, o * 4 * F:(o + cap) * 4 * F].rearrange(
                            "p (s f) -> p f s", f=F),
                        axis=AX.X, op=OP.add)
                    # add the self-loop term dinv^2 * x for this block
                    nc.vector.tensor_tensor(
                        out=agg[:], in0=agg[:],
                        in1=xddt[:, B * F:(B + 1) * F], op=OP.add)
                    pt = psump.tile([F, 128], f32, tag="pt")
                    nc.tensor.transpose(out=pt[:], in_=agg[:],
                                        identity=ident[:])
                    aggT = workp.tile([F, 128], f32, tag="aggT")
                    nc.scalar.copy(aggT[:], pt[:])
                    ph = psump.tile([128, H1], f32, tag="ph")
                    nc.tensor.matmul(out=ph[:], lhsT=aggT[:], rhs=w1[:],
                                     start=True, stop=False)
                    nc.tensor.matmul(out=ph[:], lhsT=ones_row[:], rhs=b1s[:],
                                     start=False, stop=True)
                    hd = workp.tile([128, H1], f32, tag="hd")
                    nc.scalar.activation(hd[:], ph[:], AF.Relu,
                                         scale=dinv[:, B:B + 1])
                    nc.tensor.matmul(out=psum_zT[:], lhsT=hd[:],
                                     rhs=call[:, B * G:(B + 1) * G],
                                     start=(ci == 0 and B == b0),
                                     stop=(ci == len(rep_chunks) - 1
                                           and B == b1_ - 1),
                                     skip_group_check=True)

            zT = constp.tile([H1, G], f32)
            nc.scalar.copy(zT[:], psum_zT[:])
            pP = psump.tile([G, H2], f32, tag="pP")
            nc.tensor.matmul(out=pP[:], lhsT=zT[:], rhs=w2[:],
                             start=True, stop=False)
            nc.tensor.matmul(out=pP[:], lhsT=ones_b2[:], rhs=b2s[:],
                             start=False, stop=True)
            pout = constp.tile([G, H2], f32)
            nc.scalar.activation(pout[:], pP[:], AF.Copy, scale=1.0 / GS)
            nc.sync.dma_start(out=out_t[:, :], in_=pout[:])

    nc.compile()
    return nc


_CACHE = {}


def kernel(**inputs):
    x = np.asarray(inputs["x"], dtype=np.float32)
    edge_index = np.asarray(inputs["edge_index"])
    W1 = np.asarray(inputs["W1"], dtype=np.float32)
    b1 = np.asarray(inputs["b1"], dtype=np.float32)
    W2 = np.asarray(inputs["W2"], dtype=np.float32)
    b2 = np.asarray(inputs["b2"], dtype=np.float32)
    assert x.shape == (N, F) and edge_index.shape == (2, E)

    cfg, per_core = _prep(x, edge_index, W1, b1, W2, b2)

    from concourse.bass_utils import run_bass_kernel_spmd

    if cfg not in _CACHE:
        _CACHE[cfg] = _build(cfg)
    nc = _CACHE[cfg]

    res = run_bass_kernel_spmd(nc, per_core, list(range(NCORES)))
    out = np.zeros((G, H2), np.float64)
    for r in res.results:
        out += r["p_out"].astype(np.float64)
    return out.astype(np.float32).reshape(1, G, H2)
